# revision 1
# baseline (speedup 1.0000x reference)
"""CurvatureMap Trainium2 kernel.

Computes, per batch image: channel-mean -> 3x3 Sobel-family stencils
(replicate padding) -> Gaussian/mean curvature maps (K, H, kappa).

Sharding: pure data parallel, batch b -> NeuronCore b (8 cores, full H per
core, no halo exchange).

Per-core pipeline (x_b is (64, 512, 512) f32):
  1. mean over 64 channels, one 128-row block at a time, on TensorE:
     64 accumulating float32r matmuls with stationary I/64 weights
     (identity passes the partition index through, PSUM accumulates the
     channel sum) -> mean block lands partition-aligned in PSUM.
  2. separable stencil: vertical 1D convs need row +-1 -> partition-shifted
     SBUF->SBUF DMA halo tiles; horizontal 1D convs are shifted free-axis
     slices. All conv weights are powers of two, folded into downstream
     constants.
  3. pointwise curvature math split across VectorE (fused
     scalar_tensor_tensor ops, Newton-iterated reciprocal) and ScalarE
     (Square/Sqrt/Abs with folded scales).
"""

import numpy as np

import concourse.bacc as bacc
import concourse.bass as bass
import concourse.tile as tile
from concourse import mybir
from concourse.bass_utils import run_bass_kernel_spmd

B, C, H, W = 8, 64, 512, 512
P = 128              # SBUF partitions = rows per block
NB = H // P          # row blocks per core
CHD = 8              # channels per DMA batch
Wp = W + 2           # width incl. replicate padding
F32 = mybir.dt.float32
F32R = mybir.dt.float32r
Alu = mybir.AluOpType
Act = mybir.ActivationFunctionType


def _build_nc():
    nc = bacc.Bacc()
    x_h = nc.dram_tensor("x", [C, H, W], F32R, kind="ExternalInput")
    ab_h = nc.dram_tensor("ab", [P, 2], F32, kind="ExternalInput")
    out_h = nc.dram_tensor("out", [3, H, W], F32, kind="ExternalOutput")
    eye_h = nc.inline_tensor(np.eye(P, dtype=np.float32) / C, name="eye64")

    x = x_h[:, :, :].rearrange("c h w -> h c w")
    lo = slice(0, W)
    ce = slice(1, W + 1)
    hi = slice(2, W + 2)

    with tile.TileContext(nc) as tc:
        with (
            tc.tile_pool(name="per", bufs=1) as per,
            tc.tile_pool(name="rhsp", bufs=3) as rhsp,
            tc.tile_pool(name="wk", bufs=2) as wk,
            tc.tile_pool(name="psp", bufs=4, space="PSUM") as psp,
        ):
            eye_sb = per.tile([P, P], F32R, tag="eye", bufs=1)
            nc.sync.dma_start(out=eye_sb, in_=eye_h[:, :].bitcast(F32R))
            ab_sb = per.tile([P, 2], F32, tag="ab", bufs=1)
            nc.sync.dma_start(out=ab_sb, in_=ab_h[:, :])
            alpha_col = ab_sb[:, 0:1]
            beta_col = ab_sb[:, 1:2]

            # Mean image, all 4 blocks, horizontally edge-padded.
            Mall = per.tile([P, NB, Wp], F32, tag="mall", bufs=1)

            def mean_block(k):
                ps = psp.tile([P, W], F32, tag="ps", bufs=4, name="ps")
                for g in range(C // CHD):
                    rt = rhsp.tile([P, CHD, W], F32R, tag="rhs", bufs=3, name="rt")
                    nc.sync.dma_start(
                        out=rt,
                        in_=x[k * P:(k + 1) * P, g * CHD:(g + 1) * CHD, :],
                    )
                    for ci in range(CHD):
                        ch = g * CHD + ci
                        nc.tensor.matmul(
                            ps,
                            lhsT=eye_sb,
                            rhs=rt[:, ci, :],
                            start=(ch == 0),
                            stop=(ch == C - 1),
                        )
                nc.scalar.copy(out=Mall[:, k, ce], in_=ps)
                nc.scalar.copy(out=Mall[:, k, 0:1], in_=Mall[:, k, 1:2])
                nc.scalar.copy(
                    out=Mall[:, k, W + 1:W + 2], in_=Mall[:, k, W:W + 1]
                )

            def stencil_block(k):
                T1 = Mall[:, k, :]
                # T0[p] = mean row (k*128 + p - 1), T2[p] = row (k*128 + p + 1)
                # (edge rows replicated) -- partition-shifted SBUF->SBUF DMA.
                T0 = wk.tile([P, Wp], F32, tag="T0", bufs=2)
                T2 = wk.tile([P, Wp], F32, tag="T2", bufs=2)
                up = Mall[P - 1:P, k - 1, :] if k > 0 else Mall[0:1, 0, :]
                nc.sync.dma_start(out=T0[0:1, :], in_=up)
                nc.sync.dma_start(out=T0[1:P, :], in_=Mall[0:P - 1, k, :])
                nc.sync.dma_start(out=T2[0:P - 1, :], in_=Mall[1:P, k, :])
                dn = (
                    Mall[0:1, k + 1, :]
                    if k < NB - 1
                    else Mall[P - 1:P, NB - 1, :]
                )
                nc.sync.dma_start(out=T2[P - 1:P, :], in_=dn)

                def wt(tag, w=W):
                    return wk.tile([P, w], F32, tag=tag, bufs=2, name=tag)

                # vertical 1D convs (s=[1,2,1]/4, d=[-1,0,1]/2, d2=[1,-2,1])
                # kept unscaled: Vs4 = 4*vconv_s, Bv = 2*vconv_d, Vd2 = vconv_d2
                A = wt("A", Wp)
                nc.gpsimd.tensor_add(A, T0, T2)
                Bv = wt("Bv", Wp)
                nc.gpsimd.tensor_sub(Bv, T2, T0)
                Vs4 = wt("Vs4", Wp)
                nc.vector.scalar_tensor_tensor(
                    Vs4, in0=T1, scalar=2.0, in1=A, op0=Alu.mult, op1=Alu.add
                )
                Vd2 = wt("Vd2", Wp)
                nc.vector.scalar_tensor_tensor(
                    Vd2, in0=T1, scalar=-2.0, in1=A, op0=Alu.mult, op1=Alu.add
                )

                # horizontal 1D convs, unscaled:
                # sx = 8*I_x, sy = 8*I_y, sxx = 4*I_xx, sxy = 4*I_xy, syy = 4*I_yy
                sx = wt("sx")
                nc.vector.tensor_sub(sx, Vs4[:, hi], Vs4[:, lo])
                SA = wt("SA")
                nc.vector.tensor_add(SA, Vs4[:, lo], Vs4[:, hi])
                sxx = wt("sxx")
                nc.vector.scalar_tensor_tensor(
                    sxx, in0=Vs4[:, ce], scalar=-2.0, in1=SA,
                    op0=Alu.mult, op1=Alu.add,
                )
                BA = wt("BA")
                nc.gpsimd.tensor_add(BA, Bv[:, lo], Bv[:, hi])
                sy = wt("sy")
                nc.vector.scalar_tensor_tensor(
                    sy, in0=Bv[:, ce], scalar=2.0, in1=BA,
                    op0=Alu.mult, op1=Alu.add,
                )
                sxy = wt("sxy")
                nc.vector.tensor_sub(sxy, Bv[:, hi], Bv[:, lo])
                DA = wt("DA")
                nc.gpsimd.tensor_add(DA, Vd2[:, lo], Vd2[:, hi])
                syy = wt("syy")
                nc.vector.scalar_tensor_tensor(
                    syy, in0=Vd2[:, ce], scalar=2.0, in1=DA,
                    op0=Alu.mult, op1=Alu.add,
                )

                # pointwise curvature (scales folded: Ix=sx/8, Ixx=sxx/4, ...)
                x2 = wt("x2")
                nc.scalar.activation(x2, sx, Act.Square, scale=0.125)
                y2 = wt("y2")
                nc.scalar.activation(y2, sy, Act.Square, scale=0.125)
                g_ = wt("g_")
                nc.vector.scalar_tensor_tensor(
                    g_, in0=x2, scalar=1.0, in1=y2, op0=Alu.add, op1=Alu.add
                )
                g2 = wt("g2")
                nc.scalar.activation(g2, g_, Act.Square)
                scr = wt("scr")
                rg2 = wt("rg2")
                nc.vector.reciprocal_approx_accurate(out=rg2, in_=g2, scratch=scr)
                p1 = wt("p1")
                nc.gpsimd.tensor_mul(p1, sxx, syy)
                q = wt("q")
                nc.scalar.activation(q, sxy, Act.Square, scale=0.25)
                Kn = wt("Kn")
                nc.vector.scalar_tensor_tensor(
                    Kn, in0=p1, scalar=0.0625, in1=q,
                    op0=Alu.mult, op1=Alu.subtract,
                )
                K = wt("K")
                nc.vector.tensor_mul(K, Kn, rg2)
                a1 = wt("a1")
                nc.vector.scalar_tensor_tensor(
                    a1, in0=x2, scalar=1.0, in1=syy, op0=Alu.add, op1=Alu.mult
                )
                a2 = wt("a2")
                nc.vector.scalar_tensor_tensor(
                    a2, in0=y2, scalar=1.0, in1=sxx, op0=Alu.add, op1=Alu.mult
                )
                t3 = wt("t3")
                nc.vector.tensor_add(t3, a1, a2)
                u = wt("u")
                nc.gpsimd.tensor_mul(u, sx, sy)
                v = wt("v")
                nc.vector.tensor_mul(v, u, sxy)
                Hn4 = wt("Hn4")
                nc.vector.scalar_tensor_tensor(
                    Hn4, in0=v, scalar=-0.03125, in1=t3,
                    op0=Alu.mult, op1=Alu.add,
                )
                sg = wt("sg")
                nc.scalar.activation(sg, g_, Act.Sqrt)
                m1 = wt("m1")
                nc.vector.tensor_mul(m1, Hn4, rg2)
                Hv = wt("Hv")
                nc.vector.scalar_tensor_tensor(
                    Hv, in0=m1, scalar=0.125, in1=sg, op0=Alu.mult, op1=Alu.mult
                )
                aK = wt("aK")
                nc.scalar.activation(aK, K, Act.Abs)
                aH = wt("aH")
                nc.scalar.activation(aH, Hv, Act.Abs)
                mK = wt("mK")
                nc.vector.tensor_scalar_mul(mK, aK, alpha_col)
                kap = wt("kap")
                nc.vector.scalar_tensor_tensor(
                    kap, in0=aH, scalar=beta_col, in1=mK,
                    op0=Alu.mult, op1=Alu.add,
                )

                rows = slice(k * P, (k + 1) * P)
                nc.sync.dma_start(out=out_h[0, rows, :], in_=K)
                nc.sync.dma_start(out=out_h[1, rows, :], in_=Hv)
                nc.sync.dma_start(out=out_h[2, rows, :], in_=kap)

            # Interleave: stencil(k-1) right after mean(k) so stencil DVE work
            # overlaps the remaining mean DMAs (stencil k needs means k-1..k+1).
            mean_block(0)
            for k in range(1, NB):
                mean_block(k)
                stencil_block(k - 1)
            stencil_block(NB - 1)
    return nc


_CACHE = {}


def _get_nc():
    if "nc" not in _CACHE:
        nc = _build_nc()
        nc.finalize()
        _CACHE["nc"] = nc
    return _CACHE["nc"]


def run(x, alpha, beta, **spmd_kwargs):
    x = np.ascontiguousarray(np.asarray(x, dtype=np.float32))
    assert x.shape == (B, C, H, W), x.shape
    ab = np.empty((P, 2), np.float32)
    ab[:, 0] = np.float32(alpha)
    ab[:, 1] = np.float32(beta)
    nc = _get_nc()
    in_maps = [{"x": x[b], "ab": ab} for b in range(B)]
    res = run_bass_kernel_spmd(nc, in_maps, core_ids=list(range(B)), **spmd_kwargs)
    outs = np.stack([r["out"] for r in res.results])  # (B, 3, H, W)
    K = np.ascontiguousarray(outs[:, 0:1])
    Hm = np.ascontiguousarray(outs[:, 1:2])
    kap = np.ascontiguousarray(outs[:, 2:3])
    return (K, Hm, kap), res


def kernel(x, alpha, beta):
    (K, Hm, kap), _ = run(x, alpha, beta)
    return (K, Hm, kap)



# revision 18
# speedup vs baseline: 1.0799x; 1.0799x over previous
"""CurvatureMap Trainium2 kernel.

Computes, per batch image: channel-mean -> 3x3 Sobel-family stencils
(replicate padding) -> Gaussian/mean curvature maps (K, H, kappa).

Sharding: pure data parallel, batch b -> NeuronCore b (8 cores).

Per-core pipeline, organized as 8 units = 2 column phases (j) x 4 row
blocks (k) of 128 rows; each unit covers 256 output columns and loads a
257-wide input window (1-col overlap at the phase seam), so units are
fully independent in the column direction:

  1. channel mean on TensorE: 64 accumulating f32r matmuls with a
     stationary I/64 against [128 rows, 257 cols] tiles -> PSUM, copied
     to SBUF (Act).
  2. vertical 1D convs ALSO on TensorE: tridiagonal band-matrix lhsT
     (s=[1,2,1]/4, d=[-1,0,1]/2, d2/4=[.25,-.5,.25], edge-modified
     bands at the image top/bottom) accumulated with single-element
     halo matrices reading the neighbor block's mean tile -- no halo
     DMAs.
  3. horizontal 1D convs as shifted free-axis slices of SBUF-staged
     vert-conv results, plus 1-col replicate-edge fixups at the image
     left/right border.
  4. pointwise curvature math spread across DVE/Pool/Act with conv
     scale factors folded into STT scalars / Act scales; the purely
     linear combos (g, h1, Kn) run on TensorE as scaled-identity
     accumulating matmuls into recycled PSUM banks (PE is otherwise
     idle during the tail); 1/g^2 via the single-instruction
     reciprocal_approx_fast.

Queue discipline: SP issues only input DMAs (no head-of-line blocking
on compute); consts and outputs ride the Act HWDGE.
Row-block k's stencil is emitted right after mean block k+1, and the
final two stencils are emitted interleaved (final unit first) so both
progress through the in-order engine queues -- only they trail the
input DMA stream (DMA is the roofline: 64 MiB input + 3 MiB output per
core).
"""

import numpy as np

import concourse.bacc as bacc
import concourse.bass as bass
import concourse.tile as tile
from concourse import mybir
from concourse.bass_utils import run_bass_kernel_spmd

B, C, H, W = 8, 64, 512, 512
P = 128              # SBUF partitions = rows per block
NB = H // P          # row blocks
CHD = 8              # channels per DMA batch
CW = 258             # unit input-window width (f32r matmuls need an even
                     # free size, so 256 out cols + 2 overlap cols)
OW = 256             # unit output width
WB = 255             # wide-op width (interior output columns)
F32 = mybir.dt.float32
F32R = mybir.dt.float32r
Alu = mybir.AluOpType
Act = mybir.ActivationFunctionType

# vertical kernels: s = [1,2,1]/4, d = [-1,0,1]/2, d2/4 = [1,-2,1]/4
WS = (0.25, 0.5, 0.25)
WD = (-0.5, 0.0, 0.5)
W2 = (0.25, -0.5, 0.25)


def _band(w):
    wm, w0, wp = w
    return (wm * np.eye(P, k=1) + w0 * np.eye(P) + wp * np.eye(P, k=-1)
            ).astype(np.float32)


def _build_nc():
    nc = bacc.Bacc()
    x_h = nc.dram_tensor("x", [C, H, W], F32R, kind="ExternalInput")
    ab_h = nc.dram_tensor("ab", [P, 2], F32, kind="ExternalInput")
    out_h = nc.dram_tensor("out", [3, H, W], F32, kind="ExternalOutput")
    eye_h = nc.inline_tensor(np.eye(P, dtype=np.float32) / C, name="eye64")
    bs_h = nc.inline_tensor(_band(WS), name="bands")
    bd_h = nc.inline_tensor(_band(WD), name="bandd")
    b2_h = nc.inline_tensor(_band(W2), name="band2")
    # single f32 values DMA'd into band-matrix slots (engine memsets can
    # neither start at partition 127 nor produce f32r)
    corner_vals = [WS[1] + WS[2], WD[1] + WD[2], W2[1] + W2[2],
                   WS[0], WD[0], W2[0],
                   WS[1] + WS[0], WD[1] + WD[0], W2[1] + W2[0],
                   WS[2], WD[2], W2[2]]
    cn_h = nc.inline_tensor(np.asarray([corner_vals], np.float32),
                            name="corners")

    x = x_h[:, :, :].rearrange("c h w -> h c w")
    with tile.TileContext(nc) as tc:
        with (
            tc.tile_pool(name="cs", bufs=1) as cs,
            tc.tile_pool(name="rhsp", bufs=6) as rhsp,
            tc.tile_pool(name="mp", bufs=2) as mp,
            tc.tile_pool(name="wk", bufs=2) as wk,
            tc.tile_pool(name="op", bufs=2) as op,
            tc.tile_pool(name="psp", bufs=8, space="PSUM") as psp,
        ):
            # constants + outputs ride the Act HWDGE so SP only ever
            # issues input DMAs (no head-of-line blocking of the stream)
            eye_sb = cs.tile([P, P], F32R, tag="eye", bufs=1)
            nc.scalar.dma_start(out=eye_sb, in_=eye_h[:, :].bitcast(F32R))
            ab_sb = cs.tile([P, 2], F32, tag="ab", bufs=1)
            nc.scalar.dma_start(out=ab_sb, in_=ab_h[:, :])
            alpha_col = ab_sb[:, 0:1]
            bhalf_col = ab_sb[:, 1:2]          # beta/2, folded host-side

            bsm = cs.tile([P, P], F32R, tag="bsm", bufs=1)
            nc.scalar.dma_start(out=bsm, in_=bs_h[:, :].bitcast(F32R))
            bdm = cs.tile([P, P], F32R, tag="bdm", bufs=1)
            nc.scalar.dma_start(out=bdm, in_=bd_h[:, :].bitcast(F32R))
            b2m = cs.tile([P, P], F32R, tag="b2m", bufs=1)
            nc.scalar.dma_start(out=b2m, in_=b2_h[:, :].bitcast(F32R))

            # edge-variant bands + single-element halo matrices + scaled
            # identities, built on-chip during the engine-idle startup.
            # f32r tiles may only be produced by DMA / Act / DVE, and
            # memsets can't start at partition 127 -- so all one-element
            # band edits are tiny DMAs from cn_h.
            zstage = cs.tile([P, P], F32, tag="zst", bufs=1)
            nc.vector.memset(zstage, 0.0)

            def variant(tag, src, r, c, vi):
                t = cs.tile([P, P], F32R, tag=tag, bufs=1)
                nc.scalar.copy(out=t, in_=src.bitcast(F32))
                nc.scalar.dma_start(out=t[r:r + 1, c:c + 1],
                                    in_=cn_h[0:1, vi:vi + 1].bitcast(F32R))
                return t

            def corner(tag, r, c, vi):
                t = cs.tile([P, P], F32R, tag=tag, bufs=1)
                nc.scalar.copy(out=t, in_=zstage)
                nc.scalar.dma_start(out=t[r:r + 1, c:c + 1],
                                    in_=cn_h[0:1, vi:vi + 1].bitcast(F32R))
                return t

            def scaled_eye(tag, s):
                t = cs.tile([P, P], F32R, tag=tag, bufs=1)
                nc.scalar.activation(t, eye_sb.bitcast(F32), Act.Copy,
                                     scale=s * float(C))
                return t

            bst = variant("bst", bsm, 0, 0, 6)
            bsb = variant("bsb", bsm, P - 1, P - 1, 0)
            bdt = variant("bdt", bdm, 0, 0, 7)
            bdb = variant("bdb", bdm, P - 1, P - 1, 1)
            b2t = variant("b2t", b2m, 0, 0, 8)
            b2b = variant("b2b", b2m, P - 1, P - 1, 2)
            us = corner("us", P - 1, 0, 3)   # up halo: out[0]+=wm*M_up[127]
            ds = corner("ds", 0, P - 1, 9)   # down: out[127]+=wp*M_dn[0]
            ud = corner("ud", P - 1, 0, 4)
            dd = corner("dd", 0, P - 1, 10)
            u2 = corner("u2", P - 1, 0, 5)
            d2 = corner("d2", 0, P - 1, 11)
            s_main = [bst, bsm, bsm, bsb]
            d_main = [bdt, bdm, bdm, bdb]
            v_main = [b2t, b2m, b2m, b2b]

            idn = scaled_eye("idn", 1.0)         # I
            idm = scaled_eye("idm", -1.0)        # -I

            def mean_unit(Mj, k, c0):
                ps = psp.tile([P, CW], F32, tag="ps", bufs=2, name="ps")
                for g in range(C // CHD):
                    rt = rhsp.tile([P, CHD, CW], F32R, tag="rt", bufs=6,
                                   name="rt")
                    nc.sync.dma_start(
                        out=rt,
                        in_=x[k * P:(k + 1) * P, g * CHD:(g + 1) * CHD,
                              c0:c0 + CW],
                    )
                    for ci in range(CHD):
                        ch = g * CHD + ci
                        nc.tensor.matmul(
                            ps,
                            lhsT=eye_sb,
                            rhs=rt[:, ci, :],
                            start=(ch == 0),
                            stop=(ch == C - 1),
                        )
                nc.scalar.copy(out=Mj[:, k, :], in_=ps)

            def vconv(tag, main, up, dn, Mj, k):
                """[P, CW] PSUM tile: vertical conv of the 257-col window,
                halo rows via single-element matrices reading the neighbor
                block's mean tile."""
                v = psp.tile([P, CW], F32, tag=tag, bufs=2, name=tag)
                mats = [(main[k], k)]
                if k > 0:
                    mats.append((up, k - 1))
                if k < NB - 1:
                    mats.append((dn, k + 1))
                for i, (lhsT, kk) in enumerate(mats):
                    nc.tensor.matmul(
                        v,
                        lhsT=lhsT,
                        rhs=Mj[:, kk, :],
                        start=(i == 0),
                        stop=(i == len(mats) - 1),
                    )
                return v

            def combo(tag, terms):
                """[:, 0:OW] of a recycled PSUM bank = sum of scaled SBUF
                f32r tensors, on PE via scaled-identity matmuls."""
                t = psp.tile([P, CW], F32, tag=tag, bufs=2, name=tag)
                o = t[:, 0:OW]
                for i, (idmat, s) in enumerate(terms):
                    nc.tensor.matmul(
                        o, lhsT=idmat, rhs=s,
                        start=(i == 0), stop=(i == len(terms) - 1))
                return o

            def stencil_unit(Mj, k, j):
                """Emit one unit's stencil as a list of thunks (so the final
                two units can be interleaved). Returns the thunk list."""
                steps = []
                st = {}

                def wt(tag, dt=F32):
                    return wk.tile([P, OW], dt, tag=tag, bufs=2, name=tag)

                if j == 0:
                    ws = slice(1, OW)          # wide out -> tile cols 1..255
                    e = 0                      # narrow edge at tile col 0
                    el, eh = 0, 1              # edge locals
                    off = 0                    # window local of out col 0 - 1
                else:
                    ws = slice(0, WB)          # wide out -> tile cols 0..254
                    e = OW - 1                 # narrow edge at tile col 255
                    el, eh = CW - 2, CW - 1
                    off = 1
                # ec = clamped (image-border) local col, eo = interior nbr
                ec, eo = (el, eh) if j == 0 else (eh, el)
                lo = slice(off, off + WB)
                ce = slice(off + 1, off + WB + 1)
                hi = slice(off + 2, off + WB + 2)

                def vert():
                    st["vsp"] = vconv("vs", s_main, us, ds, Mj, k)
                    st["vdp"] = vconv("vd", d_main, ud, dd, Mj, k)
                    st["v2p"] = vconv("v2", v_main, u2, d2, Mj, k)
                steps.append(vert)

                # engines may read only ONE operand from PSUM and the
                # horizontal convs pair two shifted slices -- stage the
                # vert-conv results through SBUF (Act copies)
                def vcopy():
                    for nm in ("vs", "vd", "v2"):
                        t = wk.tile([P, CW], F32, tag=nm + "b", bufs=2,
                                    name=nm + "b")
                        nc.scalar.copy(out=t, in_=st[nm + "p"])
                        st[nm] = t
                steps.append(vcopy)

                # horizontal convs; scales: sx=2*Ix, sy=4*Iy, sxx=Ixx,
                # sxy=2*Ixy, syy=Iyy
                def h1():
                    vs = st["vs"]
                    sx = st["sx"] = wt("sx")
                    nc.gpsimd.tensor_sub(
                        sx[:, e:e + 1], vs[:, eh:eh + 1], vs[:, el:el + 1])
                    nc.vector.tensor_sub(sx[:, ws], vs[:, hi], vs[:, lo])
                    SAs = st["SAs"] = wt("SAs")
                    nc.gpsimd.tensor_add(SAs[:, 0:WB], vs[:, lo], vs[:, hi])
                steps.append(h1)

                def h2():
                    vd = st["vd"]
                    SAd = st["SAd"] = wt("SAd")
                    nc.gpsimd.tensor_add(SAd[:, 0:WB], vd[:, lo], vd[:, hi])
                    sy = st["sy"] = wt("sy")
                    nc.vector.scalar_tensor_tensor(
                        sy[:, e:e + 1], in0=vd[:, ec:ec + 1], scalar=3.0,
                        in1=vd[:, eo:eo + 1], op0=Alu.mult, op1=Alu.add)
                    nc.vector.scalar_tensor_tensor(
                        sy[:, ws], in0=vd[:, ce], scalar=2.0,
                        in1=SAd[:, 0:WB], op0=Alu.mult, op1=Alu.add)
                steps.append(h2)

                def h3():
                    v2, vs = st["v2"], st["vs"]
                    SA2 = st["SA2"] = wt("SA2")
                    nc.gpsimd.tensor_add(SA2[:, 0:WB], v2[:, lo], v2[:, hi])
                    syy = st["syy"] = wt("syy")
                    nc.vector.scalar_tensor_tensor(
                        syy[:, e:e + 1], in0=v2[:, ec:ec + 1], scalar=3.0,
                        in1=v2[:, eo:eo + 1], op0=Alu.mult, op1=Alu.add)
                    nc.vector.scalar_tensor_tensor(
                        syy[:, ws], in0=v2[:, ce], scalar=2.0,
                        in1=SA2[:, 0:WB], op0=Alu.mult, op1=Alu.add)
                    sxx = st["sxx"] = wt("sxx")
                    # d2 at a replicate edge: vs[eo] - vs[ec]
                    nc.gpsimd.tensor_sub(
                        sxx[:, e:e + 1], vs[:, eo:eo + 1], vs[:, ec:ec + 1])
                    nc.vector.scalar_tensor_tensor(
                        sxx[:, ws], in0=vs[:, ce], scalar=-2.0,
                        in1=st["SAs"][:, 0:WB], op0=Alu.mult, op1=Alu.add)
                steps.append(h3)

                def h4():
                    vd = st["vd"]
                    sxy = st["sxy"] = wt("sxy")
                    nc.gpsimd.tensor_sub(
                        sxy[:, e:e + 1], vd[:, eh:eh + 1], vd[:, el:el + 1])
                    nc.vector.tensor_sub(sxy[:, ws], vd[:, hi], vd[:, lo])
                    sx2 = st["sx2"] = wt("sx2", F32R)
                    nc.scalar.activation(sx2, st["sx"], Act.Square,
                                         scale=0.5)                 # Ix^2
                    sy2 = st["sy2"] = wt("sy2", F32R)
                    nc.scalar.activation(sy2, st["sy"], Act.Square,
                                         scale=0.25)                # Iy^2
                steps.append(h4)

                def pw1():
                    # g - 1 = Ix^2 + Iy^2 on PE (the +1 folds into the
                    # Act bias of g2 and sg below)
                    st["g_"] = combo("vs", [(idn, st["sx2"]),
                                            (idn, st["sy2"])])
                    q = st["q"] = wt("q", F32R)
                    nc.scalar.activation(q, st["sxy"], Act.Square,
                                         scale=0.5)                 # Ixy^2
                    u = st["u"] = wt("u")
                    nc.gpsimd.tensor_mul(u, st["sx"], st["sy"])
                steps.append(pw1)

                def pw2():
                    g_ = st["g_"]
                    g2 = st["g2"] = wt("g2")
                    nc.scalar.activation(g2, g_, Act.Square, bias=1.0)
                    sg = st["sg"] = wt("sg")
                    nc.scalar.activation(sg, g_, Act.Sqrt, bias=1.0)
                    p1 = st["p1"] = wt("p1", F32R)
                    nc.vector.scalar_tensor_tensor(
                        p1, in0=st["sxx"], scalar=1.0, in1=st["syy"],
                        op0=Alu.mult, op1=Alu.mult)
                    a1 = st["a1"] = wt("a1", F32R)
                    nc.vector.scalar_tensor_tensor(
                        a1, in0=st["sx2"].bitcast(F32), scalar=1.0,
                        in1=st["syy"], op0=Alu.add, op1=Alu.mult)
                steps.append(pw2)

                def pw3():
                    rg2 = st["rg2"] = wt("rg2")
                    nc.vector.reciprocal_approx_fast(out=rg2, in_=st["g2"])
                    a2 = st["a2"] = wt("a2", F32R)
                    nc.vector.scalar_tensor_tensor(
                        a2, in0=st["sy2"].bitcast(F32), scalar=1.0,
                        in1=st["sxx"], op0=Alu.add, op1=Alu.mult)
                    v = st["v"] = wt("v")
                    nc.vector.tensor_mul(v, st["u"], st["sxy"])
                steps.append(pw3)

                def pw4():
                    # Kn = Ixx*Iyy - Ixy^2 ; h1c = (1+Ix^2)Iyy + (1+Iy^2)Ixx
                    st["Kn"] = combo("v2", [(idn, st["p1"]), (idm, st["q"])])
                    st["h1c"] = combo("vd", [(idn, st["a1"]),
                                             (idn, st["a2"])])
                steps.append(pw4)

                O = op.tile([P, 3, OW], F32, tag="O", bufs=2, name="O")

                def pw5():
                    Kn, rg2 = st["Kn"], st["rg2"]
                    nc.vector.tensor_mul(O[:, 0, :], Kn, rg2)       # K
                    aKn = st["aKn"] = wt("aKn")
                    nc.scalar.activation(aKn, Kn, Act.Abs)
                    Hn = st["Hn"] = wt("Hn")
                    nc.vector.scalar_tensor_tensor(
                        Hn, in0=st["v"], scalar=-0.125, in1=st["h1c"],
                        op0=Alu.mult, op1=Alu.add)
                steps.append(pw5)

                def pw6():
                    m1 = st["m1"] = wt("m1")
                    nc.vector.tensor_mul(m1, st["Hn"], st["rg2"])
                    mK1 = st["mK1"] = wt("mK1")
                    nc.gpsimd.tensor_scalar_mul(mK1, st["aKn"], alpha_col)
                    mKn = st["mKn"] = wt("mKn")
                    nc.gpsimd.tensor_mul(mKn, mK1, st["rg2"])    # alpha*|K|
                steps.append(pw6)

                def pw7():
                    m1 = st["m1"]
                    nc.vector.scalar_tensor_tensor(
                        O[:, 1, :], in0=m1, scalar=0.5, in1=st["sg"],
                        op0=Alu.mult, op1=Alu.mult)          # H
                    aHn = st["aHn"] = wt("aHn")
                    nc.scalar.activation(aHn, m1, Act.Abs)
                steps.append(pw7)

                def pw8():
                    bH1 = st["bH1"] = wt("bH1")
                    nc.gpsimd.tensor_scalar_mul(bH1, st["aHn"], bhalf_col)
                    bH = st["bH"] = wt("bH")
                    nc.gpsimd.tensor_mul(bH, bH1, st["sg"])      # beta*|H|
                steps.append(pw8)

                def fin():
                    nc.gpsimd.tensor_add(O[:, 2, :], st["mKn"], st["bH"])
                    rows = slice(k * P, (k + 1) * P)
                    oc0 = 0 if j == 0 else OW
                    nc.scalar.dma_start(
                        out=out_h[:, rows, oc0:oc0 + OW].rearrange(
                            "o h w -> h o w"),
                        in_=O,
                    )
                steps.append(fin)
                return steps

            def emit(steps):
                for s in steps:
                    s()

            for j in range(2):
                c0 = 0 if j == 0 else W - CW
                Mj = mp.tile([P, NB, CW], F32R, tag="mall", bufs=2,
                             name="mall")
                mean_unit(Mj, 0, c0)
                for k in range(1, NB - 1):
                    mean_unit(Mj, k, c0)
                    emit(stencil_unit(Mj, k - 1, j))
                mean_unit(Mj, NB - 1, c0)
                # interleave the final pair (final unit first: it is the
                # longer pole) so both chains progress concurrently
                sa = stencil_unit(Mj, NB - 2, j)
                sb = stencil_unit(Mj, NB - 1, j)
                for b_, a_ in zip(sb, sa):
                    b_()
                    a_()
    return nc


_CACHE = {}


def _get_nc():
    if "nc" not in _CACHE:
        nc = _build_nc()
        nc.finalize()
        _CACHE["nc"] = nc
    return _CACHE["nc"]


def run(x, alpha, beta, **spmd_kwargs):
    x = np.ascontiguousarray(np.asarray(x, dtype=np.float32))
    assert x.shape == (B, C, H, W), x.shape
    ab = np.empty((P, 2), np.float32)
    ab[:, 0] = np.float32(alpha)
    ab[:, 1] = np.float32(beta) * 0.5
    nc = _get_nc()
    in_maps = [{"x": x[b], "ab": ab} for b in range(B)]
    res = run_bass_kernel_spmd(nc, in_maps, core_ids=list(range(B)), **spmd_kwargs)
    outs = np.stack([r["out"] for r in res.results])  # (B, 3, H, W)
    K = np.ascontiguousarray(outs[:, 0:1])
    Hm = np.ascontiguousarray(outs[:, 1:2])
    kap = np.ascontiguousarray(outs[:, 2:3])
    return (K, Hm, kap), res


def kernel(x, alpha, beta):
    (K, Hm, kap), _ = run(x, alpha, beta)
    return (K, Hm, kap)
